# revision 6
# baseline (speedup 1.0000x reference)
# MoE (8 experts, top-2) on 8 TRN2 NeuronCores — expert-parallel.
#
# Host (numpy): router matmul + softmax + top-2 (exactly mirrors the jax
# reference arithmetic in fp32), then dispatch: gather each expert's tokens
# into a padded [D, C] column block (bf16, pre-transposed for the device
# matmul layout).
# Device (per core, expert e): hT = gelu_tanh(W1[e]^T @ xT + b1), then
# y = (hT^T @ W2[e]) * gate — all matmuls bf16 with fp32 PSUM accumulation.
# Host: scatter-add each expert's [n_e, D] result into the [N, D] output.
#
# Shapes are hardcoded for B=4, S=2048, D=1024, H=4096, E=8 (spec), but the
# builder is parametric in the padded per-expert capacity C (known only after
# routing), so the Bass program is built after routing on every call.

import numpy as np
import ml_dtypes

NUM_EXPERTS = 8
TOP_K = 2
P = 128          # SBUF partitions
TB = 512         # token block (matmul moving free size)
NOUT = 512       # output free-dim tile (one PSUM bank of fp32)

_program_cache = {}


def _build_program(C, D, H):
    import concourse.mybir as mybir
    import concourse.tile as tile
    from concourse import bacc

    bf = mybir.dt.bfloat16
    f32 = mybir.dt.float32
    Gelu = mybir.ActivationFunctionType.Gelu_apprx_tanh
    Copy = mybir.ActivationFunctionType.Copy

    KD = D // P      # contraction chunks for mm1 (8)
    KH = H // P      # contraction chunks for mm2 (32)
    ND = D // NOUT   # output column tiles (2)

    nc = bacc.Bacc(None, target_bir_lowering=False, debug=False)
    xt = nc.declare_dram_parameter("xt", [D, C], bf, isOutput=False).ap()
    w1 = nc.declare_dram_parameter("w1", [D, H], bf, isOutput=False).ap()
    w2 = nc.declare_dram_parameter("w2", [H, D], bf, isOutput=False).ap()
    g = nc.declare_dram_parameter("g", [C, 1], f32, isOutput=False).ap()
    b1t = nc.declare_dram_parameter("b1t", [P, H // P], f32, isOutput=False).ap()
    y = nc.declare_dram_parameter("y", [C, D], f32, isOutput=True).ap()

    with tile.TileContext(nc) as tc:
        with (
            tc.tile_pool(name="weights", bufs=1) as wpool,
            tc.tile_pool(name="xin", bufs=2) as xpool,
            tc.tile_pool(name="hbuf", bufs=1) as hpool,
            tc.tile_pool(name="yout", bufs=3) as ypool,
            tc.tile_pool(name="gates", bufs=3) as gpool,
            tc.tile_pool(name="ph", bufs=4, space="PSUM") as php,
            tc.tile_pool(name="py", bufs=3, space="PSUM") as pyp,
        ):
            # resident weights: W1 as KD separate [P, H] tiles (per-k dep
            # granularity so the first matmuls start as soon as their chunk
            # lands), W2 as [P, KH, D]. W2/b1 go on the ACT HWDGE ring so
            # they don't delay block-0 activations on the SP ring.
            w1_sb = [
                wpool.tile([P, H], bf, tag=f"w1sb{k}", name=f"w1sb{k}")
                for k in range(KD)
            ]
            w2_sb = wpool.tile([P, KH, D], bf, tag="w2sb")
            b1_sb = wpool.tile([P, H // P], f32, tag="b1sb")
            for k in range(KD):
                nc.sync.dma_start(w1_sb[k], w1[k * P:(k + 1) * P, :])

            nblocks = (C + TB - 1) // TB
            for b in range(nblocks):
                t0 = b * TB
                tbs = min(TB, C - t0)
                xt_blk = [
                    xpool.tile([P, tbs], bf, tag=f"xt{k}", name=f"xt{k}")
                    for k in range(KD)
                ]
                for k in range(KD):
                    nc.sync.dma_start(
                        xt_blk[k], xt[k * P:(k + 1) * P, t0:t0 + tbs]
                    )
                if b == 0:
                    for k in range(KH):
                        nc.scalar.dma_start(
                            w2_sb[:, k, :], w2[k * P:(k + 1) * P, :]
                        )
                    nc.scalar.dma_start(b1_sb, b1t)
                # mm1: hT[m] = gelu(W1_chunk^T @ xT_block + b1)  -> [P, tbs] bf16
                hT = hpool.tile([P, KH, tbs], bf, tag="hT")
                for m in range(KH):
                    ph = php.tile([P, tbs], f32, tag="ph")
                    for k in range(KD):
                        nc.tensor.matmul(
                            ph,
                            w1_sb[k][:, m * P:(m + 1) * P],
                            xt_blk[k],
                            start=(k == 0),
                            stop=(k == KD - 1),
                        )
                    nc.scalar.activation(
                        hT[:, m, :], ph, Gelu, bias=b1_sb[:, m:m + 1]
                    )
                # mm2: y[tok_tile, n] = (hT_tok^T @ W2_chunk) * gate
                for mi in range(tbs // P):
                    tok = t0 + mi * P
                    gt = gpool.tile([P, 1], f32, tag="gt")
                    nc.sync.dma_start(gt, g[tok:tok + P, :])
                    for n in range(ND):
                        py = pyp.tile([P, NOUT], f32, tag="py")
                        for k in range(KH):
                            nc.tensor.matmul(
                                py,
                                hT[:, k, mi * P:(mi + 1) * P],
                                w2_sb[:, k, n * NOUT:(n + 1) * NOUT],
                                start=(k == 0),
                                stop=(k == KH - 1),
                            )
                        # gate-multiply on DVE (keeps ACT exclusively on Gelu
                        # so its LUT tables stay warm)
                        yt = ypool.tile([P, NOUT], f32, tag="yt")
                        nc.vector.tensor_scalar_mul(yt, py, gt)
                        nc.sync.dma_start(
                            y[tok:tok + P, n * NOUT:(n + 1) * NOUT], yt
                        )
    nc.compile()
    return nc


def kernel(x, Wr, W1, b1, W2, b2):
    from concourse.bass_utils import run_bass_kernel_spmd

    bf16 = ml_dtypes.bfloat16
    B, S, D = x.shape
    E, _, H = W1.shape
    N = B * S
    xm = np.ascontiguousarray(x.reshape(N, D), dtype=np.float32)

    # --- host router (mirrors reference fp32 arithmetic; softmax is
    # monotonic so top-k on probs == top-k on logits, ties broken by index)
    logits = xm @ Wr
    mx = logits.max(axis=1, keepdims=True)
    ex = np.exp(logits - mx)
    probs = ex / ex.sum(axis=1, keepdims=True)
    top_i = np.argsort(-probs, axis=1, kind="stable")[:, :TOP_K]

    idx = [np.where((top_i == e).any(axis=1))[0] for e in range(E)]
    counts = np.array([len(i) for i in idx])
    C = max(TB, int(-(-counts.max() // P) * P))  # pad to multiple of 128

    # --- dispatch: per-expert transposed token block [D, C] bf16
    xT = np.ascontiguousarray(xm.T).astype(bf16)  # [D, N]
    in_maps = []
    for e in range(E):
        xte = np.zeros((D, C), dtype=bf16)
        xte[:, :counts[e]] = xT[:, idx[e]]
        ge = np.zeros((C, 1), dtype=np.float32)
        ge[:counts[e], 0] = probs[idx[e], e]
        in_maps.append({
            "xt": xte,
            "w1": np.ascontiguousarray(W1[e], dtype=np.float32).astype(bf16),
            "w2": np.ascontiguousarray(W2[e], dtype=np.float32).astype(bf16),
            "g": ge,
            "b1t": np.ascontiguousarray(
                np.asarray(b1[e], dtype=np.float32).reshape(H // P, P).T
            ),
        })

    key = (C, D, H)
    if key not in _program_cache:
        _program_cache[key] = _build_program(C, D, H)
    nc = _program_cache[key]

    res = run_bass_kernel_spmd(nc, in_maps, core_ids=list(range(NUM_EXPERTS)))

    # --- combine: scatter-add gated expert outputs (indices unique per expert)
    out = np.zeros((N, D), dtype=np.float32)
    b2f = np.asarray(b2, dtype=np.float32)
    for e in range(E):
        ye = np.asarray(res.results[e]["y"][:counts[e]], dtype=np.float32)
        if b2f[e].any():
            ye = ye + probs[idx[e], e][:, None] * b2f[e]
        out[idx[e]] += ye
    return out.reshape(B, S, D)


# revision 11
# speedup vs baseline: 1.2516x; 1.2516x over previous
# MoE (8 experts, top-2) on 8 TRN2 NeuronCores — expert-parallel.
#
# Host (numpy): router matmul + softmax + top-2 (exactly mirrors the jax
# reference arithmetic in fp32), then dispatch: gather each expert's tokens
# into a padded [D, C] column block (bf16, pre-transposed for the device
# matmul layout).
# Device (per core, expert e): hT = gelu_tanh(W1[e]^T @ xT + b1), then
# y = (hT^T @ W2[e]) * gate — all matmuls bf16 with fp32 PSUM accumulation.
# Host: scatter-add each expert's [n_e, D] result into the [N, D] output.
#
# Shapes are hardcoded for B=4, S=2048, D=1024, H=4096, E=8 (spec), but the
# builder is parametric in the padded per-expert capacity C (known only after
# routing), so the Bass program is built after routing on every call.

import numpy as np
import ml_dtypes

NUM_EXPERTS = 8
TOP_K = 2
P = 128          # SBUF partitions
TB = 512         # token block (matmul moving free size)
NOUT = 512       # output free-dim tile (one PSUM bank of fp32)

_program_cache = {}


def _build_program(C, D, H):
    import concourse.mybir as mybir
    import concourse.tile as tile
    from concourse import bacc

    bf = mybir.dt.bfloat16
    f32 = mybir.dt.float32
    Gelu = mybir.ActivationFunctionType.Gelu_apprx_tanh
    Copy = mybir.ActivationFunctionType.Copy

    KD = D // P      # contraction chunks for mm1 (8)
    KH = H // P      # contraction chunks for mm2 (32)
    ND = D // NOUT   # output column tiles (2)

    nc = bacc.Bacc(None, target_bir_lowering=False, debug=False)
    xt = nc.declare_dram_parameter("xt", [D, C], bf, isOutput=False).ap()
    w1 = nc.declare_dram_parameter("w1", [D, H], bf, isOutput=False).ap()
    w2 = nc.declare_dram_parameter("w2", [H, D], bf, isOutput=False).ap()
    g = nc.declare_dram_parameter("g", [C, 1], f32, isOutput=False).ap()
    b1t = nc.declare_dram_parameter("b1t", [P, H // P], f32, isOutput=False).ap()
    y = nc.declare_dram_parameter("y", [C, D], f32, isOutput=True).ap()

    with tile.TileContext(nc) as tc:
        with (
            tc.tile_pool(name="weights", bufs=1) as wpool,
            tc.tile_pool(name="xin", bufs=2) as xpool,
            tc.tile_pool(name="hbuf", bufs=1) as hpool,
            tc.tile_pool(name="yout", bufs=3) as ypool,
            tc.tile_pool(name="gates", bufs=3) as gpool,
            tc.tile_pool(name="ph", bufs=4, space="PSUM") as php,
            tc.tile_pool(name="py", bufs=3, space="PSUM") as pyp,
        ):
            # Resident weights. W1 lives as MJ column-slice tiles (each holds
            # all KD contraction chunks for a range of 4 output m-tiles) so
            # the first matmul group only waits for ~1 MiB of W1, and later
            # slices stream in behind the compute. One extra semaphore per 32
            # matmuls — per-MM wait overhead stays negligible. W2 is emitted
            # after block 0's activations in the same DMA queue (needed only
            # ~55 us in).
            MJ = 8
            HJ = H // MJ
            w1_sb = [
                wpool.tile([P, KD, HJ], bf, tag=f"w1sb{j}", name=f"w1sb{j}")
                for j in range(MJ)
            ]
            w2_sb = wpool.tile([P, KH, D], bf, tag="w2sb")
            b1_sb = wpool.tile([P, H // P], f32, tag="b1sb")
            def load_w1_slice(j):
                for k in range(KD):
                    nc.sync.dma_start(
                        w1_sb[j][:, k, :],
                        w1[k * P:(k + 1) * P, j * HJ:(j + 1) * HJ],
                    )

            nc.sync.dma_start(b1_sb, b1t)
            load_w1_slice(0)

            nblocks = (C + TB - 1) // TB
            for b in range(nblocks):
                t0 = b * TB
                tbs = min(TB, C - t0)
                xt_blk = xpool.tile([P, KD, tbs], bf, tag="xt")
                for k in range(KD):
                    nc.sync.dma_start(
                        xt_blk[:, k, :], xt[k * P:(k + 1) * P, t0:t0 + tbs]
                    )
                if b == 0:
                    for j in range(1, MJ):
                        load_w1_slice(j)
                    for k in range(KH):
                        nc.sync.dma_start(
                            w2_sb[:, k, :], w2[k * P:(k + 1) * P, :]
                        )
                # mm1: hT[m] = gelu(W1_chunk^T @ xT_block + b1)  -> [P, tbs] bf16
                hT = hpool.tile([P, KH, tbs], bf, tag="hT")
                for m in range(KH):
                    ph = php.tile([P, tbs], f32, tag="ph")
                    mj, mo = divmod(m, HJ // P)
                    for k in range(KD):
                        nc.tensor.matmul(
                            ph,
                            w1_sb[mj][:, k, mo * P:(mo + 1) * P],
                            xt_blk[:, k, :],
                            start=(k == 0),
                            stop=(k == KD - 1),
                        )
                    nc.scalar.activation(
                        hT[:, m, :], ph, Gelu, bias=b1_sb[:, m:m + 1]
                    )
                # mm2: y[tok_tile, n] = (hT_tok^T @ W2_chunk) * gate
                for mi in range(tbs // P):
                    tok = t0 + mi * P
                    gt = gpool.tile([P, 1], f32, tag="gt")
                    nc.sync.dma_start(gt, g[tok:tok + P, :])
                    for n in range(ND):
                        py = pyp.tile([P, NOUT], f32, tag="py")
                        for k in range(KH):
                            nc.tensor.matmul(
                                py,
                                hT[:, k, mi * P:(mi + 1) * P],
                                w2_sb[:, k, n * NOUT:(n + 1) * NOUT],
                                start=(k == 0),
                                stop=(k == KH - 1),
                            )
                        # gate-multiply on DVE (keeps ACT exclusively on Gelu
                        # so its LUT tables stay warm)
                        yt = ypool.tile([P, NOUT], f32, tag="yt")
                        nc.vector.tensor_scalar_mul(yt, py, gt)
                        nc.sync.dma_start(
                            y[tok:tok + P, n * NOUT:(n + 1) * NOUT], yt
                        )
    nc.compile()
    return nc


def kernel(x, Wr, W1, b1, W2, b2):
    from concourse.bass_utils import run_bass_kernel_spmd

    bf16 = ml_dtypes.bfloat16
    B, S, D = x.shape
    E, _, H = W1.shape
    N = B * S
    xm = np.ascontiguousarray(x.reshape(N, D), dtype=np.float32)

    # --- host router (mirrors reference fp32 arithmetic; softmax is
    # monotonic so top-k on probs == top-k on logits, ties broken by index)
    logits = xm @ Wr
    mx = logits.max(axis=1, keepdims=True)
    ex = np.exp(logits - mx)
    probs = ex / ex.sum(axis=1, keepdims=True)
    top_i = np.argsort(-probs, axis=1, kind="stable")[:, :TOP_K]

    idx = [np.where((top_i == e).any(axis=1))[0] for e in range(E)]
    counts = np.array([len(i) for i in idx])
    C = max(TB, int(-(-counts.max() // P) * P))  # pad to multiple of 128

    # --- dispatch: per-expert transposed token block [D, C] bf16
    xT = np.ascontiguousarray(xm.T).astype(bf16)  # [D, N]
    in_maps = []
    for e in range(E):
        xte = np.zeros((D, C), dtype=bf16)
        xte[:, :counts[e]] = xT[:, idx[e]]
        ge = np.zeros((C, 1), dtype=np.float32)
        ge[:counts[e], 0] = probs[idx[e], e]
        in_maps.append({
            "xt": xte,
            "w1": np.ascontiguousarray(W1[e], dtype=np.float32).astype(bf16),
            "w2": np.ascontiguousarray(W2[e], dtype=np.float32).astype(bf16),
            "g": ge,
            "b1t": np.ascontiguousarray(
                np.asarray(b1[e], dtype=np.float32).reshape(H // P, P).T
            ),
        })

    key = (C, D, H)
    if key not in _program_cache:
        _program_cache[key] = _build_program(C, D, H)
    nc = _program_cache[key]

    res = run_bass_kernel_spmd(nc, in_maps, core_ids=list(range(NUM_EXPERTS)))

    # --- combine: scatter-add gated expert outputs (indices unique per expert)
    out = np.zeros((N, D), dtype=np.float32)
    b2f = np.asarray(b2, dtype=np.float32)
    for e in range(E):
        ye = np.asarray(res.results[e]["y"][:counts[e]], dtype=np.float32)
        if b2f[e].any():
            ye = ye + probs[idx[e], e][:, None] * b2f[e]
        out[idx[e]] += ye
    return out.reshape(B, S, D)


# revision 16
# speedup vs baseline: 1.2643x; 1.0101x over previous
# MoE (8 experts, top-2) on 8 TRN2 NeuronCores — expert-parallel.
#
# Host (numpy): router matmul + softmax + top-2 (exactly mirrors the jax
# reference arithmetic in fp32), then dispatch: gather each expert's tokens
# into a padded [D, C] column block (bf16, pre-transposed for the device
# matmul layout).
# Device (per core, expert e): hT = gelu_tanh(W1[e]^T @ xT + b1), then
# y = (hT^T @ W2[e]) * gate — all matmuls bf16 with fp32 PSUM accumulation.
# Host: scatter-add each expert's [n_e, D] result into the [N, D] output.
#
# Shapes are hardcoded for B=4, S=2048, D=1024, H=4096, E=8 (spec), but the
# builder is parametric in the padded per-expert capacity C (known only after
# routing), so the Bass program is built after routing on every call.

import numpy as np
import ml_dtypes

NUM_EXPERTS = 8
TOP_K = 2
P = 128          # SBUF partitions
TB = 512         # token block (matmul moving free size)
NOUT = 512       # output free-dim tile (one PSUM bank of fp32)

_program_cache = {}


def _build_program(C, D, H):
    import concourse.mybir as mybir
    import concourse.tile as tile
    from concourse import bacc

    bf = mybir.dt.bfloat16
    f32 = mybir.dt.float32
    Gelu = mybir.ActivationFunctionType.Gelu_apprx_tanh

    KD = D // P      # contraction chunks for mm1 (8)
    KH = H // P      # contraction chunks for mm2 (32)
    ND = D // NOUT   # output column tiles (2)

    nc = bacc.Bacc(None, target_bir_lowering=False, debug=False)
    xt = nc.declare_dram_parameter("xt", [D, C], bf, isOutput=False).ap()
    w1 = nc.declare_dram_parameter("w1", [D, H], bf, isOutput=False).ap()
    w2 = nc.declare_dram_parameter("w2", [H, D], bf, isOutput=False).ap()
    g = nc.declare_dram_parameter("g", [C, 1], f32, isOutput=False).ap()
    b1t = nc.declare_dram_parameter("b1t", [P, H // P], f32, isOutput=False).ap()
    y = nc.declare_dram_parameter("y", [C, D], f32, isOutput=True).ap()

    with tile.TileContext(nc) as tc:
        with (
            tc.tile_pool(name="weights", bufs=1) as wpool,
            tc.tile_pool(name="xin", bufs=2) as xpool,
            tc.tile_pool(name="hbuf", bufs=1) as hpool,
            tc.tile_pool(name="yout", bufs=3) as ypool,
            tc.tile_pool(name="ph", bufs=5, space="PSUM") as php,
            tc.tile_pool(name="py", bufs=3, space="PSUM") as pyp,
        ):
            # Resident weights. W1 lives as MJ column-slice tiles (each holds
            # all KD contraction chunks for a range of 4 output m-tiles) so
            # the first matmul group only waits for ~1 MiB of W1, and later
            # slices stream in behind the compute. One extra semaphore per 32
            # matmuls — per-MM wait overhead stays negligible. W2 is emitted
            # after block 0's activations in the same DMA queue (needed only
            # ~55 us in).
            MJ = 8
            HJ = H // MJ
            w1_sb = [
                wpool.tile([P, KD, HJ], bf, tag=f"w1sb{j}", name=f"w1sb{j}")
                for j in range(MJ)
            ]
            w2_sb = wpool.tile([P, KH, D], bf, tag="w2sb")
            b1_sb = wpool.tile([P, H // P], f32, tag="b1sb")
            gs_sb = wpool.tile([P, C // P], f32, tag="gssb")

            # partition-major DRAM views so each load is ONE multi-dim DMA
            # (fewer per-transfer first-byte latencies during startup)
            w1_r = w1.rearrange("(k p) h -> p k h", p=P)
            w2_r = w2.rearrange("(k p) d -> p k d", p=P)
            xt_r = xt.rearrange("(k p) c -> p k c", p=P)
            g_r = g.rearrange("(t p) one -> p t one", p=P)

            nc.sync.dma_start(b1_sb, b1t)
            nc.sync.dma_start(gs_sb, g_r[:, :, 0])
            nc.sync.dma_start(w1_sb[0], w1_r[:, :, 0:HJ])

            nblocks = (C + TB - 1) // TB
            for b in range(nblocks):
                t0 = b * TB
                tbs = min(TB, C - t0)
                xt_blk = xpool.tile([P, KD, tbs], bf, tag="xt")
                nc.sync.dma_start(xt_blk, xt_r[:, :, t0:t0 + tbs])
                if b == 0:
                    for j in range(1, MJ):
                        nc.sync.dma_start(
                            w1_sb[j], w1_r[:, :, j * HJ:(j + 1) * HJ]
                        )
                    nc.sync.dma_start(w2_sb, w2_r)
                # mm1: hT[m] = gelu(W1_chunk^T @ xT_block + b1)  -> [P, tbs] bf16
                hT = hpool.tile([P, KH, tbs], bf, tag="hT")
                for m in range(KH):
                    ph = php.tile([P, tbs], f32, tag="ph")
                    mj, mo = divmod(m, HJ // P)
                    for k in range(KD):
                        nc.tensor.matmul(
                            ph,
                            w1_sb[mj][:, k, mo * P:(mo + 1) * P],
                            xt_blk[:, k, :],
                            start=(k == 0),
                            stop=(k == KD - 1),
                        )
                    nc.scalar.activation(
                        hT[:, m, :], ph, Gelu, bias=b1_sb[:, m:m + 1]
                    )
                # mm2: y[tok_tile] = (hT_tok^T @ W2) * gate
                for mi in range(tbs // P):
                    tok = t0 + mi * P
                    yt = ypool.tile([P, D], f32, tag="yt")
                    for n in range(ND):
                        py = pyp.tile([P, NOUT], f32, tag="py")
                        for k in range(KH):
                            nc.tensor.matmul(
                                py,
                                hT[:, k, mi * P:(mi + 1) * P],
                                w2_sb[:, k, n * NOUT:(n + 1) * NOUT],
                                start=(k == 0),
                                stop=(k == KH - 1),
                            )
                        # gate-multiply on DVE (keeps ACT exclusively on Gelu
                        # so its LUT tables stay warm)
                        nc.vector.tensor_scalar_mul(
                            yt[:, n * NOUT:(n + 1) * NOUT],
                            py,
                            gs_sb[:, tok // P:tok // P + 1],
                        )
                    nc.sync.dma_start(y[tok:tok + P, :], yt)
    nc.compile()
    return nc


def _ensure_trace_hooks():
    # bass_utils' trace path (taken when BASS_TRACE=1 is set externally)
    # imports antenv.axon_hooks, which this image lacks. Shim it (and the
    # artifact upload, which needs a bucket) only when missing, so tracing
    # degrades gracefully instead of crashing.
    import sys
    import types

    try:
        import antenv.axon_hooks  # noqa: F401
        return
    except ImportError:
        pass
    try:
        import antenv

        mod = types.ModuleType("antenv.axon_hooks")
        state = {"hook": None}
        mod.set_axon_ntff_profile_hook = lambda h: state.__setitem__("hook", h)
        mod.get_axon_ntff_profile_hook = lambda: state["hook"]
        sys.modules["antenv.axon_hooks"] = mod
        antenv.axon_hooks = mod
        try:
            from trn_agent_boot.trn_boot import _ntff_profile_via_ctypes

            mod.set_axon_ntff_profile_hook(
                _ntff_profile_via_ctypes("/opt/axon/libaxon_pjrt.so")
            )
            import concourse.bass_utils as _bu

            _orig_upload = _bu.upload_artifacts

            def _safe_upload(tmpdir):
                try:
                    return _orig_upload(tmpdir)
                except Exception:
                    return f"local:{tmpdir}"

            _bu.upload_artifacts = _safe_upload
        except Exception:
            pass
    except Exception:
        pass


def kernel(x, Wr, W1, b1, W2, b2):
    _ensure_trace_hooks()
    from concourse.bass_utils import run_bass_kernel_spmd

    bf16 = ml_dtypes.bfloat16
    B, S, D = x.shape
    E, _, H = W1.shape
    N = B * S
    xm = np.ascontiguousarray(x.reshape(N, D), dtype=np.float32)

    # --- host router (mirrors reference fp32 arithmetic; softmax is
    # monotonic so top-k on probs == top-k on logits, ties broken by index)
    logits = xm @ Wr
    mx = logits.max(axis=1, keepdims=True)
    ex = np.exp(logits - mx)
    probs = ex / ex.sum(axis=1, keepdims=True)
    top_i = np.argsort(-probs, axis=1, kind="stable")[:, :TOP_K]

    idx = [np.where((top_i == e).any(axis=1))[0] for e in range(E)]
    counts = np.array([len(i) for i in idx])
    C = max(TB, int(-(-counts.max() // P) * P))  # pad to multiple of 128

    # --- dispatch: per-expert transposed token block [D, C] bf16
    xT = np.ascontiguousarray(xm.T).astype(bf16)  # [D, N]
    in_maps = []
    for e in range(E):
        xte = np.zeros((D, C), dtype=bf16)
        xte[:, :counts[e]] = xT[:, idx[e]]
        ge = np.zeros((C, 1), dtype=np.float32)
        ge[:counts[e], 0] = probs[idx[e], e]
        in_maps.append({
            "xt": xte,
            "w1": np.ascontiguousarray(W1[e], dtype=np.float32).astype(bf16),
            "w2": np.ascontiguousarray(W2[e], dtype=np.float32).astype(bf16),
            "g": ge,
            "b1t": np.ascontiguousarray(
                np.asarray(b1[e], dtype=np.float32).reshape(H // P, P).T
            ),
        })

    key = (C, D, H)
    if key not in _program_cache:
        _program_cache[key] = _build_program(C, D, H)
    nc = _program_cache[key]

    res = run_bass_kernel_spmd(nc, in_maps, core_ids=list(range(NUM_EXPERTS)))

    # --- combine: scatter-add gated expert outputs (indices unique per expert)
    out = np.zeros((N, D), dtype=np.float32)
    b2f = np.asarray(b2, dtype=np.float32)
    for e in range(E):
        ye = np.asarray(res.results[e]["y"][:counts[e]], dtype=np.float32)
        if b2f[e].any():
            ye = ye + probs[idx[e], e][:, None] * b2f[e]
        out[idx[e]] += ye
    return out.reshape(B, S, D)
